# revision 19
# baseline (speedup 1.0000x reference)
"""Trainium2 Bass kernel for the CherryAllocation NAGNN (grid GIN + MLP head).

Self-contained: hardcodes shapes/sharding. Data-parallel over batch:
64 samples -> 8 NeuronCores x 8 samples. Weights replicated.

Math per sample (grid 32x32, N=1024 nodes):
  mask = obs[:1024] != 0 ; x = obs[1024:].reshape(1024, 32)
  h0 = x
  for l in 0..3:  agg = sum of 4-neighbor h ; h = relu(LN(agg @ Wl + bl) * g + be)
  xc = concat([x, h1, h2, h3, h4])  # [1024, 1056]
  z  = relu(BN(xc @ W1 + b1))       # BN eval-mode affine
  y  = z @ W2 + b2 ; out = where(mask, y, -1e7)

v3 design notes:
 - all matmul operands bf16 (PSUM accumulates fp32).
 - h stored feature-major with a fully padded grid: each grid row padded to
   34 cols (1 zero on each side), plus one zero pad-row above and below
   (chunk width 34*34=1156). The 4-neighbor aggregation is then 3 plain
   strided adds (no edge fixups), split between GpSimd and DVE, into a
   compact agg tile that feeds single-pass layer matmuls (act-stationary).
 - pads are zeroed only on each pool slot's first use; later writes only
   touch token cells, so pads stay zero.
 - LayerNorm without mean: weights are column-mean-centered on the host so
   z has exactly zero feature-mean. Per block: variance via DVE
   bn_stats/bn_aggr (blocks 0-5) or ACT Square+accum (6-7); sqrt+recip per
   block-pair; normalize = scale-only (DVE tensor_scalar blocks 0-1, ACT
   elsewhere) into bf16 t_nm; relu is applied by the post-transpose
   PSUM->SBUF move (valid since gamma==1, beta==0 here; the general
   affine path falls back to ACT affine+relu after the transpose).
 - samples processed in groups of 4; group 0's W1/W2 interleaves with
   group 1's layer phases to keep the PE fed during LN chains.
"""

import numpy as np

import concourse.bass as bass
import concourse.bacc as bacc
import concourse.mybir as mybir
import concourse.tile as tile
from concourse.bass_utils import run_bass_kernel_spmd
from concourse.masks import make_identity

FP = mybir.dt.float32
BF = mybir.dt.bfloat16
AF = mybir.ActivationFunctionType
OP = mybir.AluOpType

GRID = 32
NN = 1024            # nodes per sample
F_IN = 32
H = 256
B = 64
S = 8                # samples per core
NCORE = 8
NB = 8               # 128-token blocks per sample
OBS_W = NN + NN * F_IN   # 33792
MIN_VAL = -10000000.0
EPS_LN = 1e-5
EPS_BN = 1e-5
RW = GRID + 2        # padded grid-row width (34)
CW = RW * (GRID + 2)  # padded chunk width (34*34 = 1156)
G = 4                # samples per pipeline group

USE_BF16 = True
PROFILE = False
LAST_EXEC_NS = None
TRACE_KWARGS = {}

# Engine-balance knobs (aff_trivial path).
STATS_ACT = (6, 7)     # blocks using ACT Square+accum for variance
NORM_DVE = (0, 1)      # blocks normalized on DVE (others on ACT)
MOVE2_DVE = {(0, 0)}   # (half, c) move2 copies on DVE instead of ACT


def _build(has_gin_bias: bool, b2_val: float, aff_trivial: bool) -> bass.Bass:
    nc = bacc.Bacc("TRN2", target_bir_lowering=False, debug=False)

    obs = nc.declare_dram_parameter("obs", [S, OBS_W], FP, isOutput=False)
    w0 = nc.declare_dram_parameter("w0", [F_IN, H], BF, isOutput=False)
    ws = nc.declare_dram_parameter("ws", [3, 2, 128, H], BF, isOutput=False)
    w1x = nc.declare_dram_parameter("w1x", [F_IN, 512], BF, isOutput=False)
    w1h = nc.declare_dram_parameter("w1h", [8, 128, 512], BF, isOutput=False)
    w2 = nc.declare_dram_parameter("w2", [4, 128], BF, isOutput=False)
    if not aff_trivial:
        gg = nc.declare_dram_parameter("gg", [4, H], FP, isOutput=False)
        bb = nc.declare_dram_parameter("bb", [4, H], FP, isOutput=False)
    bns = nc.declare_dram_parameter("bns", [512], FP, isOutput=False)
    bnt = nc.declare_dram_parameter("bnt", [512], FP, isOutput=False)
    if has_gin_bias:
        gbias = nc.declare_dram_parameter("gbias", [4, H], FP, isOutput=False)
    y_out = nc.declare_dram_parameter("y", [S, NN], FP, isOutput=True)

    from contextlib import ExitStack

    with tile.TileContext(nc) as tc, ExitStack() as ctx:
        wp = ctx.enter_context(tc.tile_pool(name="w", bufs=1))
        px = ctx.enter_context(tc.tile_pool(name="px", bufs=2))
        ph = ctx.enter_context(tc.tile_pool(name="ph", bufs=2))
        pst = ctx.enter_context(tc.tile_pool(name="pst", bufs=8))
        pfin = ctx.enter_context(tc.tile_pool(name="pfin", bufs=1))
        pz = ctx.enter_context(tc.tile_pool(name="pz", bufs=5, space="PSUM"))
        ptf = ctx.enter_context(tc.tile_pool(name="ptf", bufs=2, space="PSUM"))
        pyp = ctx.enter_context(tc.tile_pool(name="pyp", bufs=1, space="PSUM"))

        # ---- constants / weights in SBUF ----
        ident = wp.tile([128, 128], BF, tag="id")
        make_identity(nc, ident[:])
        eps_sb = wp.tile([128, 1], FP, tag="eps")
        nc.gpsimd.memset(eps_sb[:], EPS_LN)

        # Per-sample zero-padded W0: quadrant q holds W0, other rows zero, so
        # group-packed x operands (sample i at partitions 32i..32i+32) can
        # contract over all 128 partitions with base partition 0.
        w0_sb = []
        for q in range(4):
            t = wp.tile([128, H], BF, tag=f"w0q{q}")
            nc.vector.memset(t[:].bitcast(mybir.dt.uint16), 0)
            nc.sync.dma_start(t[32 * q: 32 * (q + 1), :], w0[:, :])
            w0_sb.append(t)

        wl_sb = []
        for l in range(3):
            t = wp.tile([128, 2 * H], BF, tag=f"wl{l}")
            nc.sync.dma_start(
                t[:].rearrange("p (k n) -> p k n", k=2),
                ws[l].rearrange("k p n -> p k n"),
            )
            wl_sb.append(t)

        w1x_sb = []
        for q in range(4):
            t = wp.tile([128, 512], BF, tag=f"w1xq{q}")
            nc.vector.memset(t[:].bitcast(mybir.dt.uint16), 0)
            nc.sync.dma_start(t[32 * q: 32 * (q + 1), :], w1x[:, :])
            w1x_sb.append(t)
        w1h_sb = wp.tile([128, 8 * 512], BF, tag="w1h")
        nc.sync.dma_start(
            w1h_sb[:].rearrange("p (j m) -> p j m", j=8),
            w1h[:, :, :].rearrange("j p m -> p j m"),
        )
        w2_sb = wp.tile([128, 4], BF, tag="w2")
        nc.sync.dma_start(w2_sb[:], w2[:, :].rearrange("k p -> p k"))

        if not aff_trivial:
            gg_sb = wp.tile([128, 8], FP, tag="gg")
            nc.sync.dma_start(
                gg_sb[:].rearrange("p (l c) -> p l c", c=2),
                gg[:, :].rearrange("l (c p) -> p l c", p=128),
            )
            bb_sb = wp.tile([128, 8], FP, tag="bb")
            nc.sync.dma_start(
                bb_sb[:].rearrange("p (l c) -> p l c", c=2),
                bb[:, :].rearrange("l (c p) -> p l c", p=128),
            )
        bns_sb = wp.tile([128, 4], FP, tag="bns")
        nc.sync.dma_start(bns_sb[:], bns[:].rearrange("(m p) -> p m", p=128))
        bnt_sb = wp.tile([128, 4], FP, tag="bnt")
        nc.sync.dma_start(bnt_sb[:], bnt[:].rearrange("(m p) -> p m", p=128))

        if has_gin_bias:
            ones1 = wp.tile([1, 128], BF, tag="ones1")
            nc.gpsimd.memset(ones1[:].bitcast(mybir.dt.uint16), 0x3F80)
            gb_sb = wp.tile([1, 4 * H], BF, tag="gb")
            nc.gpsimd.dma_start(
                gb_sb[:].rearrange("q (l n) -> q l n", l=4), gbias[:, :]
            )

        def grid_view(t, kc, dr, dc):
            """[128, 32, 32] token view of padded tile t, shifted (dr,dc)."""
            v = t[:, kc * CW: (kc + 1) * CW].rearrange(
                "p (r c) -> p r c", c=RW)
            return v[:, 1 + dr: 33 + dr, 1 + dc: 33 + dc]

        _ms_alt = [0]

        def init_pads(t, n_kc):
            GI = mybir.dt.uint16
            for kc in range(n_kc):
                eng = nc.vector if _ms_alt[0] % 2 else nc.gpsimd
                _ms_alt[0] += 1
                eng.memset(t[:, kc * CW: kc * CW + RW].bitcast(GI), 0)
                eng.memset(
                    t[:, (kc + 1) * CW - RW: (kc + 1) * CW].bitcast(GI), 0)
                side = t[:, kc * CW + RW: kc * CW + RW + GRID * RW].rearrange(
                    "p (r c) -> p r c", c=RW)
                eng.memset(side[:, :, 0:1].bitcast(GI), 0)
                eng.memset(side[:, :, 33:34].bitcast(GI), 0)

        def prep_dma(g):
            """Issue the x-feature loads for samples 4g..4g+3 (cast DMAs)."""
            x_nm4 = px.tile([128, 4 * 256], BF, tag="xnm", bufs=2)
            for si in range(G):
                s = G * g + si
                nc.gpsimd.dma_start(
                    x_nm4[:, si * 256: (si + 1) * 256].rearrange(
                        "p (b f) -> p b f", f=F_IN),
                    obs[s, NN:OBS_W].rearrange("(b p f) -> p b f", p=128, f=F_IN),
                )
            return x_nm4

        # Group 0's x DMAs go out before the ~100 pad memsets below so the
        # first PE transposes aren't stuck behind them on the GpSimd queue.
        x_nm4_g0 = prep_dma(0)

        # h and x FM tiles live in a manual ring (pads zeroed exactly once;
        # token-cell writers never touch pads, so pads stay zero; Tile's
        # dependency tracking handles WAR on slot reuse).
        xfm_slots = []
        for k in range(2):
            t = wp.tile([128, CW], BF, tag=f"xfms{k}")
            init_pads(t, 1)
            xfm_slots.append(t)
        _x_cnt = [0]
        h_slots = []
        for l in range(4):
            row = []
            for k in range(6):
                t = wp.tile([128, 2 * CW], BF, tag=f"h{l}s{k}")
                init_pads(t, 2)
                row.append(t)
            h_slots.append(row)
        _h_cnt = [0, 0, 0, 0]

        def build_agg_chunk(agg_tile, src_tile, kc, tmp_tile, gps_final):
            """agg = h[left]+h[right]+h[up]+h[down] for one chunk; add tree
            so the GpSimd and DVE halves run in parallel before combining.
            gps_final puts the combine on GpSimd — right when DVE is the
            regional bottleneck (group 0's solo phases), wrong when DVE has
            slack (the W1-filler region)."""
            av = agg_tile[:, kc * NN: (kc + 1) * NN].rearrange(
                "p (r c) -> p r c", c=GRID)
            tv = tmp_tile[:, kc * NN: (kc + 1) * NN].rearrange(
                "p (r c) -> p r c", c=GRID)
            nc.gpsimd.tensor_add(tv, grid_view(src_tile, kc, 0, -1),
                                 grid_view(src_tile, kc, 0, 1))
            nc.vector.tensor_add(av, grid_view(src_tile, kc, -1, 0),
                                 grid_view(src_tile, kc, 1, 0))
            eng = nc.gpsimd if gps_final else nc.vector
            eng.tensor_add(av, av, tv)

        def build_agg(agg_tile, src_tile, n_kc, tmp_tile, gps_final=True):
            for kc in range(n_kc):
                build_agg_chunk(agg_tile, src_tile, kc, tmp_tile, gps_final)

        def prep_group(g, x_nm4=None):
            """Transpose x for samples 4g..4g+3 into FM, partition-packed."""
            if x_nm4 is None:
                x_nm4 = prep_dma(g)
            x4_fm = xfm_slots[_x_cnt[0] % 2]
            _x_cnt[0] += 1
            for si in range(G):
                for half in range(2):
                    x_tfm = ptf.tile([F_IN, 512], BF, tag="tf")
                    for i in range(4):
                        b = half * 4 + i
                        nc.tensor.transpose(
                            x_tfm[:, i * 128: (i + 1) * 128],
                            x_nm4[:, si * 256 + b * F_IN: si * 256 + (b + 1) * F_IN],
                            ident[:],
                        )
                    dst = grid_view(x4_fm, 0, 0, 0)[
                        32 * si: 32 * (si + 1),
                        16 * half: 16 * (half + 1), :]
                    nc.scalar.copy(
                        dst,
                        x_tfm[:].rearrange("p (r c) -> p r c", c=GRID),
                    )
            agg_x4 = px.tile([128, NN], BF, tag="aggx", bufs=2)
            tmpx = px.tile([128, NN], BF, tag="tmpx", bufs=2)
            build_agg(agg_x4, x4_fm, 1, tmpx)
            return [
                {"s": G * g + si, "si": si, "x4_fm": x4_fm, "agg_x4": agg_x4,
                 "h": []}
                for si in range(G)
            ]

        def layer_mm_phase(st, l):
            """Matmuls + LN stats + normalize -> t_nm (node-major)."""
            si = st["si"]
            # mean/var per block interleaved: mv16[:, 2b]=mean, [:, 2b+1]=var
            mv16 = pst.tile([128, 16], FP, tag="var")
            t_nm = ph.tile([128, NB * H], BF, tag="tnm", bufs=3)
            for hb in range(2):
                z2s = []
                for bp in (2 * hb, 2 * hb + 1):
                    z2 = pz.tile([128, 512], FP, tag="z")
                    z2s.append(z2)
                    for i in range(2):
                        b = 2 * bp + i
                        zc = z2[:, i * 256: (i + 1) * 256]
                        if l == 0:
                            ax = st["agg_x4"]
                            lhsT = ax[:, b * 128: (b + 1) * 128]
                            rhs = w0_sb[si][:, :]
                            n = 1 + (1 if has_gin_bias else 0)
                            nc.tensor.matmul(zc, lhsT, rhs,
                                             start=True, stop=(n == 1))
                        else:
                            agg = st["agg"]
                            wl = wl_sb[l - 1]
                            n = 2 + (1 if has_gin_bias else 0)
                            for kc in range(2):
                                nc.tensor.matmul(
                                    zc,
                                    agg[:, kc * NN + b * 128: kc * NN + (b + 1) * 128],
                                    wl[:, kc * H: (kc + 1) * H],
                                    start=(kc == 0), stop=(kc == n - 1),
                                )
                        if has_gin_bias:
                            nc.tensor.matmul(
                                zc, ones1[0:1, 0:128],
                                gb_sb[0:1, l * H: (l + 1) * H],
                                start=False, stop=True,
                            )
                        # variance (weights mean-centered on host => mean==0)
                        if b in STATS_ACT:
                            # tensor output is scratch; lands in the t_nm
                            # slot that the normalize below overwrites.
                            nc.scalar.activation(
                                t_nm[:, b * H: (b + 1) * H], zc, AF.Square,
                                bias=0.0, scale=0.0625,  # (z/16)^2 = z^2/256
                                accum_out=mv16[:, 2 * b + 1: 2 * b + 2],
                            )
                        else:
                            st6 = pst.tile([128, 6], FP, tag="st6")
                            nc.vector.bn_stats(st6[:], zc)
                            nc.vector.bn_aggr(mv16[:, 2 * b: 2 * b + 2],
                                              st6[:])
                # per-half sqrt + reciprocal (batched over 4 blocks)
                varh = mv16[:].rearrange("p (b t) -> p t b", t=2)[:, 1,
                                                                  4 * hb: 4 * hb + 4]
                sdh = pst.tile([128, 4], FP, tag="sd")
                nc.scalar.activation(sdh[:], varh, AF.Sqrt,
                                     bias=eps_sb[:, 0:1], scale=1.0)
                invh = pst.tile([128, 4], FP, tag="inv")
                nc.vector.reciprocal(invh[:], sdh[:])
                for j in range(4):
                    b = 4 * hb + j
                    dst = t_nm[:, b * H: (b + 1) * H]
                    zc = z2s[j // 2][:, (j % 2) * 256: (j % 2 + 1) * 256]
                    if aff_trivial and b in NORM_DVE:
                        nc.vector.tensor_scalar(
                            dst, zc, invh[:, j: j + 1], None, OP.mult)
                    else:
                        nc.scalar.activation(
                            dst, zc, AF.Copy, bias=0.0,
                            scale=invh[:, j: j + 1])
            st["t_nm"] = t_nm

        def layer_tr_phase(st, l):
            """Transpose t_nm -> FM h (relu applied by the PSUM->SBUF move).
            Chunk-major order: each h chunk's aggregation is emitted as soon
            as its two copies land, overlapping the other chunk's work."""
            t_nm = st.pop("t_nm")
            h_t = h_slots[l][_h_cnt[l] % 6]
            _h_cnt[l] += 1
            if l < 3:
                agg_t = ph.tile([128, 2 * NN], BF, tag="agg", bufs=3)
                tmp_t = ph.tile([128, 2 * NN], BF, tag="tmpa", bufs=2)
            for c in range(2):
                for half in range(2):
                    tf = ptf.tile([128, 512], BF, tag="tf")
                    for i in range(4):
                        b = half * 4 + i
                        nc.tensor.transpose(
                            tf[:, i * 128: (i + 1) * 128],
                            t_nm[:, b * H + c * 128: b * H + c * 128 + 128],
                            ident[:],
                        )
                    dst = grid_view(h_t, c, 0, 0)[
                        :, 16 * half: 16 * (half + 1), :]
                    tfv = tf[:].rearrange("p (r c) -> p r c", c=GRID)
                    if not aff_trivial:
                        nc.scalar.activation(
                            dst, tfv, AF.Relu,
                            scale=gg_sb[:, l * 2 + c: l * 2 + c + 1],
                            bias=bb_sb[:, l * 2 + c: l * 2 + c + 1],
                        )
                    elif (half, c) in MOVE2_DVE:
                        nc.vector.tensor_relu(dst, tfv)
                    else:
                        nc.scalar.activation(dst, tfv, AF.Relu)
                if l < 3:
                    build_agg_chunk(agg_t, h_t, c, tmp_t,
                                    gps_final=(st["s"] < G))
            st["h"].append(h_t)
            if l < 3:
                st["agg"] = agg_t

        def unit_w1(st, mc_list=None):
            si = st["si"]
            if "z_sb" in st:
                z_sb = st["z_sb"]
            else:
                z_sb = ph.tile([128, 4096], BF, tag="zsb", bufs=2)
                st["z_sb"] = z_sb
            for m, c2 in (mc_list if mc_list is not None
                          else [(m, c2) for m in range(4) for c2 in range(2)]):
                    zw1 = ptf.tile([128, 512], FP, tag="tf")
                    for kc in range(9):
                        if kc == 0:
                            lhsT = w1x_sb[si][:, m * 128: (m + 1) * 128]
                            rhs = grid_view(st["x4_fm"], 0, 0, 0)[
                                :, 16 * c2: 16 * (c2 + 1), :]
                        else:
                            j = kc - 1
                            lhsT = w1h_sb[:, j * 512 + m * 128
                                          : j * 512 + (m + 1) * 128]
                            rhs = grid_view(st["h"][j // 2], j % 2, 0, 0)[
                                :, 16 * c2: 16 * (c2 + 1), :]
                        nc.tensor.matmul(
                            zw1[:, :], lhsT, rhs,
                            start=(kc == 0), stop=(kc == 8),
                        )
                    nc.scalar.activation(
                        z_sb[:, m * NN + c2 * 512: m * NN + (c2 + 1) * 512],
                        zw1[:],
                        AF.Relu,
                        scale=bns_sb[:, m: m + 1],
                        bias=bnt_sb[:, m: m + 1],
                    )
            st["z_sb"] = z_sb

        def unit_w2(st):
            s = st["s"]
            z_sb = st.pop("z_sb")
            y_s = pfin.tile([1, NN], FP, tag="ys", bufs=1)
            for c2 in range(2):
                yp = pyp.tile([1, 512], FP, tag="yp")
                for m in range(4):
                    nc.tensor.matmul(
                        yp[0:1, :],
                        w2_sb[:, m: m + 1],
                        z_sb[:, m * NN + c2 * 512: m * NN + (c2 + 1) * 512],
                        start=(m == 0), stop=(m == 3),
                    )
                nc.vector.tensor_copy(y_s[:, c2 * 512: (c2 + 1) * 512],
                                      yp[0:1, :])
            if b2_val != 0.0:
                nc.scalar.add(y_s[:], y_s[:], b2_val)
            m_s = pfin.tile([1, NN], FP, tag="ms", bufs=1)
            nc.sync.dma_start(m_s[:], obs[s: s + 1, 0:NN])
            yf = pfin.tile([1, NN], FP, tag="yfin", bufs=1)
            nc.gpsimd.memset(yf[:], MIN_VAL)
            nc.vector.copy_predicated(yf[:], m_s[:].bitcast(mybir.dt.uint32),
                                      y_s[:])
            nc.sync.dma_start(y_out[s: s + 1, :], yf[:])

        # ---- emission schedule ----
        # group 0 layers; then group 1 layers with group 0's W1/W2
        # interleaved as PE filler; group 1's W1/W2 tail.
        MC_A = [(0, 0), (0, 1), (1, 0), (1, 1)]
        MC_B = [(2, 0), (2, 1), (3, 0), (3, 1)]
        sts0 = prep_group(0, x_nm4_g0)
        for l in range(4):
            for st in sts0:
                layer_mm_phase(st, l)
            if l == 0:
                sts1 = prep_group(1)
            for st in sts0:
                layer_tr_phase(st, l)
        # G1 layer-l tr phases overwrite h slots of G0's s0/s1 (ring reuse),
        # so those two samples' W1 must fully precede tr(s6)/tr(s7) at l=0.
        for l in range(4):
            for st in sts1:
                layer_mm_phase(st, l)
            if l == 0:
                unit_w1(sts0[0], MC_A)
                layer_tr_phase(sts1[0], l)
                unit_w1(sts0[0], MC_B)
                layer_tr_phase(sts1[1], l)
                unit_w1(sts0[1], MC_A)
                layer_tr_phase(sts1[2], l)
                unit_w1(sts0[1], MC_B)
                layer_tr_phase(sts1[3], l)
                unit_w2(sts0[0])
                unit_w2(sts0[1])
            elif l == 1:
                unit_w1(sts0[2], MC_A)
                for st in sts1[:2]:
                    layer_tr_phase(st, l)
                unit_w1(sts0[2], MC_B)
                for st in sts1[2:]:
                    layer_tr_phase(st, l)
                unit_w2(sts0[2])
            elif l == 2:
                unit_w1(sts0[3], MC_A)
                for st in sts1[:2]:
                    layer_tr_phase(st, l)
                for st in sts1[2:]:
                    layer_tr_phase(st, l)
            else:  # l == 3: remaining G0 filler, then per-sample tail
                unit_w1(sts0[3], MC_B)
                unit_w2(sts0[3])
                for st in sts1:
                    layer_tr_phase(st, l)
                    unit_w1(st)
        for st in sts1:
            unit_w2(st)

    nc.finalize()
    return nc


_BUILD_CACHE = {}


def _get_nc(has_gin_bias: bool, b2_val: float, aff_trivial: bool) -> bass.Bass:
    key = (has_gin_bias, float(b2_val), aff_trivial)
    if key not in _BUILD_CACHE:
        _BUILD_CACHE[key] = _build(has_gin_bias, b2_val, aff_trivial)
    return _BUILD_CACHE[key]


def prep_maps(observations, W0, b0, g0, be0, Ws, bs, gs, bes,
              W1, b1, bn_g, bn_b, bn_m, bn_v, W2, b2, **_ignored):
    obs = np.ascontiguousarray(np.asarray(observations, np.float32))
    W0 = np.asarray(W0, np.float64)
    Ws = np.asarray(Ws, np.float64)
    W1 = np.asarray(W1, np.float32)
    W2 = np.asarray(W2, np.float32)
    gg = np.ascontiguousarray(np.stack(
        [np.asarray(g0, np.float32)] + [np.asarray(gs, np.float32)[i] for i in range(3)]))
    bb = np.ascontiguousarray(np.stack(
        [np.asarray(be0, np.float32)] + [np.asarray(bes, np.float32)[i] for i in range(3)]))
    gbias = np.stack(
        [np.asarray(b0, np.float64)] + [np.asarray(bs, np.float64)[i] for i in range(3)])
    has_gin_bias = bool(np.any(gbias != 0.0))
    aff_trivial = bool(np.all(gg == 1.0) and np.all(bb == 0.0))
    bn_scale = (np.asarray(bn_g, np.float32)
                / np.sqrt(np.asarray(bn_v, np.float32) + EPS_BN)).astype(np.float32)
    bn_shift = ((np.asarray(b1, np.float32) - np.asarray(bn_m, np.float32)) * bn_scale
                + np.asarray(bn_b, np.float32)).astype(np.float32)
    b2_val = float(np.asarray(b2, np.float32).reshape(-1)[0])

    # Center GIN weights so z = agg @ W' has exactly zero feature-mean
    # (LayerNorm then needs no mean subtraction).
    W0c = (W0 - W0.mean(axis=1, keepdims=True)).astype(np.float32)
    Wsc = (Ws - Ws.mean(axis=2, keepdims=True)).astype(np.float32)
    gbias_c = (gbias - gbias.mean(axis=1, keepdims=True)).astype(np.float32)

    import ml_dtypes
    BFD = ml_dtypes.bfloat16
    ws_r = np.ascontiguousarray(Wsc.reshape(3, 2, 128, H).astype(BFD))
    w1x = np.ascontiguousarray(W1[:F_IN].astype(BFD))
    w1h = np.ascontiguousarray(W1[F_IN:].reshape(8, 128, 512).astype(BFD))
    w2r = np.ascontiguousarray(W2.reshape(4, 128).astype(BFD))
    W0c = W0c.astype(BFD)

    shared = {
        "w0": np.ascontiguousarray(W0c), "ws": ws_r, "w1x": w1x, "w1h": w1h,
        "w2": w2r, "bns": bn_scale, "bnt": bn_shift,
    }
    if not aff_trivial:
        shared["gg"] = gg
        shared["bb"] = bb
    if has_gin_bias:
        shared["gbias"] = np.ascontiguousarray(gbias_c)
    in_maps = []
    for c in range(NCORE):
        m = dict(shared)
        m["obs"] = np.ascontiguousarray(obs[c * S: (c + 1) * S])
        in_maps.append(m)
    return in_maps, has_gin_bias, b2_val, aff_trivial


def kernel(**inputs) -> np.ndarray:
    global LAST_EXEC_NS
    in_maps, has_gin_bias, b2_val, aff_trivial = prep_maps(**inputs)
    nc = _get_nc(has_gin_bias, b2_val, aff_trivial)
    res = run_bass_kernel_spmd(
        nc, in_maps, list(range(NCORE)), trace=PROFILE, **TRACE_KWARGS
    )
    LAST_EXEC_NS = res.exec_time_ns
    y = np.concatenate([res.results[c]["y"] for c in range(NCORE)], axis=0)
    return y.reshape(B, NN).astype(np.float32)


# revision 20
# speedup vs baseline: 1.0170x; 1.0170x over previous
"""Trainium2 Bass kernel for the CherryAllocation NAGNN (grid GIN + MLP head).

Self-contained: hardcodes shapes/sharding. Data-parallel over batch:
64 samples -> 8 NeuronCores x 8 samples. Weights replicated.

Math per sample (grid 32x32, N=1024 nodes):
  mask = obs[:1024] != 0 ; x = obs[1024:].reshape(1024, 32)
  h0 = x
  for l in 0..3:  agg = sum of 4-neighbor h ; h = relu(LN(agg @ Wl + bl) * g + be)
  xc = concat([x, h1, h2, h3, h4])  # [1024, 1056]
  z  = relu(BN(xc @ W1 + b1))       # BN eval-mode affine
  y  = z @ W2 + b2 ; out = where(mask, y, -1e7)

v3 design notes:
 - all matmul operands bf16 (PSUM accumulates fp32).
 - h stored feature-major with a fully padded grid: each grid row padded to
   34 cols (1 zero on each side), plus one zero pad-row above and below
   (chunk width 34*34=1156). The 4-neighbor aggregation is then 3 plain
   strided adds (no edge fixups), split between GpSimd and DVE, into a
   compact agg tile that feeds single-pass layer matmuls (act-stationary).
 - pads are zeroed only on each pool slot's first use; later writes only
   touch token cells, so pads stay zero.
 - LayerNorm without mean: weights are column-mean-centered on the host so
   z has exactly zero feature-mean. Per block: variance via DVE
   bn_stats/bn_aggr (blocks 0-5) or ACT Square+accum (6-7); sqrt+recip per
   block-pair; normalize = scale-only (DVE tensor_scalar blocks 0-1, ACT
   elsewhere) into bf16 t_nm; relu is applied by the post-transpose
   PSUM->SBUF move (valid since gamma==1, beta==0 here; the general
   affine path falls back to ACT affine+relu after the transpose).
 - samples processed in groups of 4; group 0's W1/W2 interleaves with
   group 1's layer phases to keep the PE fed during LN chains.
"""

import numpy as np

import concourse.bass as bass
import concourse.bacc as bacc
import concourse.mybir as mybir
import concourse.tile as tile
from concourse.bass_utils import run_bass_kernel_spmd
from concourse.masks import make_identity

FP = mybir.dt.float32
BF = mybir.dt.bfloat16
AF = mybir.ActivationFunctionType
OP = mybir.AluOpType

GRID = 32
NN = 1024            # nodes per sample
F_IN = 32
H = 256
B = 64
S = 8                # samples per core
NCORE = 8
NB = 8               # 128-token blocks per sample
OBS_W = NN + NN * F_IN   # 33792
MIN_VAL = -10000000.0
EPS_LN = 1e-5
EPS_BN = 1e-5
RW = GRID + 2        # padded grid-row width (34)
CW = RW * (GRID + 2)  # padded chunk width (34*34 = 1156)
G = 4                # samples per pipeline group

USE_BF16 = True
PROFILE = False
LAST_EXEC_NS = None
TRACE_KWARGS = {}

# Engine-balance knobs (aff_trivial path).
STATS_ACT = (6, 7)     # blocks using ACT Square+accum for variance
NORM_DVE = (0, 1)      # blocks normalized on DVE (others on ACT)
MOVE2_DVE = {(0, 0)}   # (half, c) move2 copies on DVE instead of ACT


def _build(has_gin_bias: bool, b2_val: float, aff_trivial: bool) -> bass.Bass:
    nc = bacc.Bacc("TRN2", target_bir_lowering=False, debug=False)

    obs = nc.declare_dram_parameter("obs", [S, OBS_W], FP, isOutput=False)
    w0 = nc.declare_dram_parameter("w0", [F_IN, H], BF, isOutput=False)
    ws = nc.declare_dram_parameter("ws", [3, 2, 128, H], BF, isOutput=False)
    w1x = nc.declare_dram_parameter("w1x", [F_IN, 512], BF, isOutput=False)
    w1h = nc.declare_dram_parameter("w1h", [8, 128, 512], BF, isOutput=False)
    w2 = nc.declare_dram_parameter("w2", [4, 128], BF, isOutput=False)
    if not aff_trivial:
        gg = nc.declare_dram_parameter("gg", [4, H], FP, isOutput=False)
        bb = nc.declare_dram_parameter("bb", [4, H], FP, isOutput=False)
    bns = nc.declare_dram_parameter("bns", [512], FP, isOutput=False)
    bnt = nc.declare_dram_parameter("bnt", [512], FP, isOutput=False)
    if has_gin_bias:
        gbias = nc.declare_dram_parameter("gbias", [4, H], FP, isOutput=False)
    y_out = nc.declare_dram_parameter("y", [S, NN], FP, isOutput=True)

    from contextlib import ExitStack

    with tile.TileContext(nc) as tc, ExitStack() as ctx:
        wp = ctx.enter_context(tc.tile_pool(name="w", bufs=1))
        px = ctx.enter_context(tc.tile_pool(name="px", bufs=2))
        ph = ctx.enter_context(tc.tile_pool(name="ph", bufs=2))
        pst = ctx.enter_context(tc.tile_pool(name="pst", bufs=8))
        pfin = ctx.enter_context(tc.tile_pool(name="pfin", bufs=1))
        pz = ctx.enter_context(tc.tile_pool(name="pz", bufs=5, space="PSUM"))
        ptf = ctx.enter_context(tc.tile_pool(name="ptf", bufs=2, space="PSUM"))
        pyp = ctx.enter_context(tc.tile_pool(name="pyp", bufs=1, space="PSUM"))

        # ---- constants / weights in SBUF ----
        ident = wp.tile([128, 128], BF, tag="id")
        make_identity(nc, ident[:])
        eps_sb = wp.tile([128, 1], FP, tag="eps")
        nc.gpsimd.memset(eps_sb[:], EPS_LN)

        # Per-sample zero-padded W0: quadrant q holds W0, other rows zero, so
        # group-packed x operands (sample i at partitions 32i..32i+32) can
        # contract over all 128 partitions with base partition 0.
        w0_sb = []
        for q in range(4):
            t = wp.tile([128, H], BF, tag=f"w0q{q}")
            nc.vector.memset(t[:].bitcast(mybir.dt.uint16), 0)
            nc.sync.dma_start(t[32 * q: 32 * (q + 1), :], w0[:, :])
            w0_sb.append(t)

        wl_sb = []
        for l in range(3):
            t = wp.tile([128, 2 * H], BF, tag=f"wl{l}")
            nc.sync.dma_start(
                t[:].rearrange("p (k n) -> p k n", k=2),
                ws[l].rearrange("k p n -> p k n"),
            )
            wl_sb.append(t)

        w1x_sb = []
        for q in range(4):
            t = wp.tile([128, 512], BF, tag=f"w1xq{q}")
            nc.vector.memset(t[:].bitcast(mybir.dt.uint16), 0)
            nc.sync.dma_start(t[32 * q: 32 * (q + 1), :], w1x[:, :])
            w1x_sb.append(t)
        w1h_sb = wp.tile([128, 8 * 512], BF, tag="w1h")
        nc.sync.dma_start(
            w1h_sb[:].rearrange("p (j m) -> p j m", j=8),
            w1h[:, :, :].rearrange("j p m -> p j m"),
        )
        w2_sb = wp.tile([128, 4], BF, tag="w2")
        nc.sync.dma_start(w2_sb[:], w2[:, :].rearrange("k p -> p k"))

        if not aff_trivial:
            gg_sb = wp.tile([128, 8], FP, tag="gg")
            nc.sync.dma_start(
                gg_sb[:].rearrange("p (l c) -> p l c", c=2),
                gg[:, :].rearrange("l (c p) -> p l c", p=128),
            )
            bb_sb = wp.tile([128, 8], FP, tag="bb")
            nc.sync.dma_start(
                bb_sb[:].rearrange("p (l c) -> p l c", c=2),
                bb[:, :].rearrange("l (c p) -> p l c", p=128),
            )
        bns_sb = wp.tile([128, 4], FP, tag="bns")
        nc.sync.dma_start(bns_sb[:], bns[:].rearrange("(m p) -> p m", p=128))
        bnt_sb = wp.tile([128, 4], FP, tag="bnt")
        nc.sync.dma_start(bnt_sb[:], bnt[:].rearrange("(m p) -> p m", p=128))

        if has_gin_bias:
            ones1 = wp.tile([1, 128], BF, tag="ones1")
            nc.gpsimd.memset(ones1[:].bitcast(mybir.dt.uint16), 0x3F80)
            gb_sb = wp.tile([1, 4 * H], BF, tag="gb")
            nc.gpsimd.dma_start(
                gb_sb[:].rearrange("q (l n) -> q l n", l=4), gbias[:, :]
            )

        def grid_view(t, kc, dr, dc):
            """[128, 32, 32] token view of padded tile t, shifted (dr,dc)."""
            v = t[:, kc * CW: (kc + 1) * CW].rearrange(
                "p (r c) -> p r c", c=RW)
            return v[:, 1 + dr: 33 + dr, 1 + dc: 33 + dc]

        _ms_alt = [0]

        def init_pads(t, n_kc):
            GI = mybir.dt.uint16
            for kc in range(n_kc):
                eng = nc.vector if _ms_alt[0] % 2 else nc.gpsimd
                _ms_alt[0] += 1
                eng.memset(t[:, kc * CW: kc * CW + RW].bitcast(GI), 0)
                eng.memset(
                    t[:, (kc + 1) * CW - RW: (kc + 1) * CW].bitcast(GI), 0)
                side = t[:, kc * CW + RW: kc * CW + RW + GRID * RW].rearrange(
                    "p (r c) -> p r c", c=RW)
                eng.memset(side[:, :, 0:1].bitcast(GI), 0)
                eng.memset(side[:, :, 33:34].bitcast(GI), 0)

        def prep_dma(g):
            """Issue the x-feature loads for samples 4g..4g+3 (cast DMAs)."""
            x_nm4 = px.tile([128, 4 * 256], BF, tag="xnm", bufs=2)
            for si in range(G):
                s = G * g + si
                nc.gpsimd.dma_start(
                    x_nm4[:, si * 256: (si + 1) * 256].rearrange(
                        "p (b f) -> p b f", f=F_IN),
                    obs[s, NN:OBS_W].rearrange("(b p f) -> p b f", p=128, f=F_IN),
                )
            return x_nm4

        # Group 0's x DMAs go out before the ~100 pad memsets below so the
        # first PE transposes aren't stuck behind them on the GpSimd queue.
        x_nm4_g0 = prep_dma(0)

        # h and x FM tiles live in a manual ring (pads zeroed exactly once;
        # token-cell writers never touch pads, so pads stay zero; Tile's
        # dependency tracking handles WAR on slot reuse).
        xfm_slots = []
        for k in range(2):
            t = wp.tile([128, CW], BF, tag=f"xfms{k}")
            init_pads(t, 1)
            xfm_slots.append(t)
        _x_cnt = [0]
        h_slots = []
        for l in range(4):
            row = []
            for k in range(6):
                t = wp.tile([128, 2 * CW], BF, tag=f"h{l}s{k}")
                init_pads(t, 2)
                row.append(t)
            h_slots.append(row)
        _h_cnt = [0, 0, 0, 0]

        def build_agg_chunk(agg_tile, src_tile, kc, tmp_tile):
            """agg = h[left]+h[right]+h[up]+h[down] for one chunk; add tree
            so the GpSimd and DVE halves run in parallel before combining."""
            av = agg_tile[:, kc * NN: (kc + 1) * NN].rearrange(
                "p (r c) -> p r c", c=GRID)
            tv = tmp_tile[:, kc * NN: (kc + 1) * NN].rearrange(
                "p (r c) -> p r c", c=GRID)
            nc.gpsimd.tensor_add(tv, grid_view(src_tile, kc, 0, -1),
                                 grid_view(src_tile, kc, 0, 1))
            nc.vector.tensor_add(av, grid_view(src_tile, kc, -1, 0),
                                 grid_view(src_tile, kc, 1, 0))
            nc.vector.tensor_add(av, av, tv)

        def build_agg(agg_tile, src_tile, n_kc, tmp_tile):
            for kc in range(n_kc):
                build_agg_chunk(agg_tile, src_tile, kc, tmp_tile)

        def prep_group(g, x_nm4=None):
            """Transpose x for samples 4g..4g+3 into FM, partition-packed."""
            if x_nm4 is None:
                x_nm4 = prep_dma(g)
            x4_fm = xfm_slots[_x_cnt[0] % 2]
            _x_cnt[0] += 1
            for si in range(G):
                for half in range(2):
                    x_tfm = ptf.tile([F_IN, 512], BF, tag="tf")
                    for i in range(4):
                        b = half * 4 + i
                        nc.tensor.transpose(
                            x_tfm[:, i * 128: (i + 1) * 128],
                            x_nm4[:, si * 256 + b * F_IN: si * 256 + (b + 1) * F_IN],
                            ident[:],
                        )
                    dst = grid_view(x4_fm, 0, 0, 0)[
                        32 * si: 32 * (si + 1),
                        16 * half: 16 * (half + 1), :]
                    nc.scalar.copy(
                        dst,
                        x_tfm[:].rearrange("p (r c) -> p r c", c=GRID),
                    )
            agg_x4 = px.tile([128, NN], BF, tag="aggx", bufs=2)
            tmpx = px.tile([128, NN], BF, tag="tmpx", bufs=2)
            build_agg(agg_x4, x4_fm, 1, tmpx)
            return [
                {"s": G * g + si, "si": si, "x4_fm": x4_fm, "agg_x4": agg_x4,
                 "h": []}
                for si in range(G)
            ]

        def layer_mm_phase(st, l):
            """Matmuls + LN stats + normalize -> t_nm (node-major)."""
            si = st["si"]
            # mean/var per block interleaved: mv16[:, 2b]=mean, [:, 2b+1]=var
            mv16 = pst.tile([128, 16], FP, tag="var")
            t_nm = ph.tile([128, NB * H], BF, tag="tnm", bufs=3)
            for hb in range(2):
                z2s = []
                for bp in (2 * hb, 2 * hb + 1):
                    z2 = pz.tile([128, 512], FP, tag="z")
                    z2s.append(z2)
                    for i in range(2):
                        b = 2 * bp + i
                        zc = z2[:, i * 256: (i + 1) * 256]
                        if l == 0:
                            ax = st["agg_x4"]
                            lhsT = ax[:, b * 128: (b + 1) * 128]
                            rhs = w0_sb[si][:, :]
                            n = 1 + (1 if has_gin_bias else 0)
                            nc.tensor.matmul(zc, lhsT, rhs,
                                             start=True, stop=(n == 1))
                        else:
                            agg = st["agg"]
                            wl = wl_sb[l - 1]
                            n = 2 + (1 if has_gin_bias else 0)
                            for kc in range(2):
                                nc.tensor.matmul(
                                    zc,
                                    agg[:, kc * NN + b * 128: kc * NN + (b + 1) * 128],
                                    wl[:, kc * H: (kc + 1) * H],
                                    start=(kc == 0), stop=(kc == n - 1),
                                )
                        if has_gin_bias:
                            nc.tensor.matmul(
                                zc, ones1[0:1, 0:128],
                                gb_sb[0:1, l * H: (l + 1) * H],
                                start=False, stop=True,
                            )
                        # variance (weights mean-centered on host => mean==0)
                        if b in STATS_ACT:
                            # tensor output is scratch; lands in the t_nm
                            # slot that the normalize below overwrites.
                            nc.scalar.activation(
                                t_nm[:, b * H: (b + 1) * H], zc, AF.Square,
                                bias=0.0, scale=0.0625,  # (z/16)^2 = z^2/256
                                accum_out=mv16[:, 2 * b + 1: 2 * b + 2],
                            )
                        else:
                            st6 = pst.tile([128, 6], FP, tag="st6")
                            nc.vector.bn_stats(st6[:], zc)
                            nc.vector.bn_aggr(mv16[:, 2 * b: 2 * b + 2],
                                              st6[:])
                # per-half sqrt + reciprocal (batched over 4 blocks)
                varh = mv16[:].rearrange("p (b t) -> p t b", t=2)[:, 1,
                                                                  4 * hb: 4 * hb + 4]
                sdh = pst.tile([128, 4], FP, tag="sd")
                nc.scalar.activation(sdh[:], varh, AF.Sqrt,
                                     bias=eps_sb[:, 0:1], scale=1.0)
                invh = pst.tile([128, 4], FP, tag="inv")
                nc.vector.reciprocal(invh[:], sdh[:])
                for j in range(4):
                    b = 4 * hb + j
                    dst = t_nm[:, b * H: (b + 1) * H]
                    zc = z2s[j // 2][:, (j % 2) * 256: (j % 2 + 1) * 256]
                    if aff_trivial and b in NORM_DVE:
                        nc.vector.tensor_scalar(
                            dst, zc, invh[:, j: j + 1], None, OP.mult)
                    else:
                        nc.scalar.activation(
                            dst, zc, AF.Copy, bias=0.0,
                            scale=invh[:, j: j + 1])
            st["t_nm"] = t_nm

        def layer_tr_phase(st, l):
            """Transpose t_nm -> FM h (relu applied by the PSUM->SBUF move).
            Chunk-major order: each h chunk's aggregation is emitted as soon
            as its two copies land, overlapping the other chunk's work."""
            t_nm = st.pop("t_nm")
            h_t = h_slots[l][_h_cnt[l] % 6]
            _h_cnt[l] += 1
            if l < 3:
                agg_t = ph.tile([128, 2 * NN], BF, tag="agg", bufs=3)
                tmp_t = ph.tile([128, 2 * NN], BF, tag="tmpa", bufs=2)
            for c in range(2):
                for half in range(2):
                    tf = ptf.tile([128, 512], BF, tag="tf")
                    for i in range(4):
                        b = half * 4 + i
                        nc.tensor.transpose(
                            tf[:, i * 128: (i + 1) * 128],
                            t_nm[:, b * H + c * 128: b * H + c * 128 + 128],
                            ident[:],
                        )
                    dst = grid_view(h_t, c, 0, 0)[
                        :, 16 * half: 16 * (half + 1), :]
                    tfv = tf[:].rearrange("p (r c) -> p r c", c=GRID)
                    if not aff_trivial:
                        nc.scalar.activation(
                            dst, tfv, AF.Relu,
                            scale=gg_sb[:, l * 2 + c: l * 2 + c + 1],
                            bias=bb_sb[:, l * 2 + c: l * 2 + c + 1],
                        )
                    elif (half, c) in MOVE2_DVE:
                        nc.vector.tensor_relu(dst, tfv)
                    else:
                        nc.scalar.activation(dst, tfv, AF.Relu)
                if l < 3:
                    build_agg_chunk(agg_t, h_t, c, tmp_t)
            st["h"].append(h_t)
            if l < 3:
                st["agg"] = agg_t

        def unit_w1(st, mc_list=None):
            si = st["si"]
            if "z_sb" in st:
                z_sb = st["z_sb"]
            else:
                z_sb = ph.tile([128, 4096], BF, tag="zsb", bufs=2)
                st["z_sb"] = z_sb
            for m, c2 in (mc_list if mc_list is not None
                          else [(m, c2) for m in range(4) for c2 in range(2)]):
                    zw1 = ptf.tile([128, 512], FP, tag="tf")
                    for kc in range(9):
                        if kc == 0:
                            lhsT = w1x_sb[si][:, m * 128: (m + 1) * 128]
                            rhs = grid_view(st["x4_fm"], 0, 0, 0)[
                                :, 16 * c2: 16 * (c2 + 1), :]
                        else:
                            j = kc - 1
                            lhsT = w1h_sb[:, j * 512 + m * 128
                                          : j * 512 + (m + 1) * 128]
                            rhs = grid_view(st["h"][j // 2], j % 2, 0, 0)[
                                :, 16 * c2: 16 * (c2 + 1), :]
                        nc.tensor.matmul(
                            zw1[:, :], lhsT, rhs,
                            start=(kc == 0), stop=(kc == 8),
                        )
                    nc.scalar.activation(
                        z_sb[:, m * NN + c2 * 512: m * NN + (c2 + 1) * 512],
                        zw1[:],
                        AF.Relu,
                        scale=bns_sb[:, m: m + 1],
                        bias=bnt_sb[:, m: m + 1],
                    )
            st["z_sb"] = z_sb

        def unit_w2(st):
            s = st["s"]
            z_sb = st.pop("z_sb")
            y_s = pfin.tile([1, NN], FP, tag="ys", bufs=1)
            for c2 in range(2):
                yp = pyp.tile([1, 512], FP, tag="yp")
                for m in range(4):
                    nc.tensor.matmul(
                        yp[0:1, :],
                        w2_sb[:, m: m + 1],
                        z_sb[:, m * NN + c2 * 512: m * NN + (c2 + 1) * 512],
                        start=(m == 0), stop=(m == 3),
                    )
                nc.vector.tensor_copy(y_s[:, c2 * 512: (c2 + 1) * 512],
                                      yp[0:1, :])
            if b2_val != 0.0:
                nc.scalar.add(y_s[:], y_s[:], b2_val)
            m_s = pfin.tile([1, NN], FP, tag="ms", bufs=1)
            nc.sync.dma_start(m_s[:], obs[s: s + 1, 0:NN])
            yf = pfin.tile([1, NN], FP, tag="yfin", bufs=1)
            nc.gpsimd.memset(yf[:], MIN_VAL)
            nc.vector.copy_predicated(yf[:], m_s[:].bitcast(mybir.dt.uint32),
                                      y_s[:])
            nc.sync.dma_start(y_out[s: s + 1, :], yf[:])

        # ---- emission schedule ----
        # group 0 layers; then group 1 layers with group 0's W1/W2
        # interleaved as PE filler; group 1's W1/W2 tail.
        MC_A = [(0, 0), (0, 1), (1, 0), (1, 1)]
        MC_B = [(2, 0), (2, 1), (3, 0), (3, 1)]
        sts0 = prep_group(0, x_nm4_g0)
        for l in range(4):
            for st in sts0:
                layer_mm_phase(st, l)
            if l == 0:
                sts1 = prep_group(1)
            for st in sts0:
                layer_tr_phase(st, l)
        # G1 layer-l tr phases overwrite h slots of G0's s0/s1 (ring reuse),
        # so those two samples' W1 must fully precede tr(s6)/tr(s7) at l=0.
        for l in range(4):
            for st in sts1:
                layer_mm_phase(st, l)
            if l == 0:
                unit_w1(sts0[0])
                layer_tr_phase(sts1[0], l)
                layer_tr_phase(sts1[1], l)
                unit_w1(sts0[1])
                layer_tr_phase(sts1[2], l)
                layer_tr_phase(sts1[3], l)
                unit_w2(sts0[0])
                unit_w2(sts0[1])
            elif l == 1:
                unit_w1(sts0[2], MC_A)
                for st in sts1[:2]:
                    layer_tr_phase(st, l)
                unit_w1(sts0[2], MC_B)
                for st in sts1[2:]:
                    layer_tr_phase(st, l)
                unit_w2(sts0[2])
            elif l == 2:
                unit_w1(sts0[3], MC_A)
                for st in sts1[:2]:
                    layer_tr_phase(st, l)
                for st in sts1[2:]:
                    layer_tr_phase(st, l)
            else:  # l == 3: remaining G0 filler, then per-sample tail
                unit_w1(sts0[3], MC_B)
                unit_w2(sts0[3])
                for st in sts1:
                    layer_tr_phase(st, l)
                    unit_w1(st)
        for st in sts1:
            unit_w2(st)

    nc.finalize()
    return nc


_BUILD_CACHE = {}


def _get_nc(has_gin_bias: bool, b2_val: float, aff_trivial: bool) -> bass.Bass:
    key = (has_gin_bias, float(b2_val), aff_trivial)
    if key not in _BUILD_CACHE:
        _BUILD_CACHE[key] = _build(has_gin_bias, b2_val, aff_trivial)
    return _BUILD_CACHE[key]


def prep_maps(observations, W0, b0, g0, be0, Ws, bs, gs, bes,
              W1, b1, bn_g, bn_b, bn_m, bn_v, W2, b2, **_ignored):
    obs = np.ascontiguousarray(np.asarray(observations, np.float32))
    W0 = np.asarray(W0, np.float64)
    Ws = np.asarray(Ws, np.float64)
    W1 = np.asarray(W1, np.float32)
    W2 = np.asarray(W2, np.float32)
    gg = np.ascontiguousarray(np.stack(
        [np.asarray(g0, np.float32)] + [np.asarray(gs, np.float32)[i] for i in range(3)]))
    bb = np.ascontiguousarray(np.stack(
        [np.asarray(be0, np.float32)] + [np.asarray(bes, np.float32)[i] for i in range(3)]))
    gbias = np.stack(
        [np.asarray(b0, np.float64)] + [np.asarray(bs, np.float64)[i] for i in range(3)])
    has_gin_bias = bool(np.any(gbias != 0.0))
    aff_trivial = bool(np.all(gg == 1.0) and np.all(bb == 0.0))
    bn_scale = (np.asarray(bn_g, np.float32)
                / np.sqrt(np.asarray(bn_v, np.float32) + EPS_BN)).astype(np.float32)
    bn_shift = ((np.asarray(b1, np.float32) - np.asarray(bn_m, np.float32)) * bn_scale
                + np.asarray(bn_b, np.float32)).astype(np.float32)
    b2_val = float(np.asarray(b2, np.float32).reshape(-1)[0])

    # Center GIN weights so z = agg @ W' has exactly zero feature-mean
    # (LayerNorm then needs no mean subtraction).
    W0c = (W0 - W0.mean(axis=1, keepdims=True)).astype(np.float32)
    Wsc = (Ws - Ws.mean(axis=2, keepdims=True)).astype(np.float32)
    gbias_c = (gbias - gbias.mean(axis=1, keepdims=True)).astype(np.float32)

    import ml_dtypes
    BFD = ml_dtypes.bfloat16
    ws_r = np.ascontiguousarray(Wsc.reshape(3, 2, 128, H).astype(BFD))
    w1x = np.ascontiguousarray(W1[:F_IN].astype(BFD))
    w1h = np.ascontiguousarray(W1[F_IN:].reshape(8, 128, 512).astype(BFD))
    w2r = np.ascontiguousarray(W2.reshape(4, 128).astype(BFD))
    W0c = W0c.astype(BFD)

    shared = {
        "w0": np.ascontiguousarray(W0c), "ws": ws_r, "w1x": w1x, "w1h": w1h,
        "w2": w2r, "bns": bn_scale, "bnt": bn_shift,
    }
    if not aff_trivial:
        shared["gg"] = gg
        shared["bb"] = bb
    if has_gin_bias:
        shared["gbias"] = np.ascontiguousarray(gbias_c)
    in_maps = []
    for c in range(NCORE):
        m = dict(shared)
        m["obs"] = np.ascontiguousarray(obs[c * S: (c + 1) * S])
        in_maps.append(m)
    return in_maps, has_gin_bias, b2_val, aff_trivial


def kernel(**inputs) -> np.ndarray:
    global LAST_EXEC_NS
    in_maps, has_gin_bias, b2_val, aff_trivial = prep_maps(**inputs)
    nc = _get_nc(has_gin_bias, b2_val, aff_trivial)
    res = run_bass_kernel_spmd(
        nc, in_maps, list(range(NCORE)), trace=PROFILE, **TRACE_KWARGS
    )
    LAST_EXEC_NS = res.exec_time_ns
    y = np.concatenate([res.results[c]["y"] for c in range(NCORE)], axis=0)
    return y.reshape(B, NN).astype(np.float32)
